# revision 9
# baseline (speedup 1.0000x reference)
"""Trainium2 Bass kernel for nn_Attention_51127290692370.

Dense transformer attention block:
    q = LN(x @ Wq) ; k = LN(x @ Wk) ; v = x @ Wv        (LN over full D=1024)
    out = softmax(q_h @ k_h^T) @ v_h  per head (16 heads, hd=64, scale 1.0)
    return out @ Wo

Sharding over 8 NeuronCores: core c handles batch b=c//4 and query-block
j=c%4 (512 of 2048 rows). The host rotates x[b] by 512*j rows so the SPMD
program is identical on every core (own queries are always rotated rows
0..511); softmax/PV are permutation-invariant over the key order, so k/v
built from the rotated x give the same output rows.

Numerics: q/k projections in fp32r (TF32-class); scores, PV, and the output
projection in bf16 with fp32 PSUM accumulation. Softmax skips the max
subtraction (scores for this problem are in [-70, 63] => exp stays in fp32
range) and normalization is deferred: PV's stationary operand carries a ones
column so each head's psum holds [outT_unnorm; row_sums]; outT is divided by
the sums right before the Wo projection.
"""

import numpy as np

import concourse.bass as bass
import concourse.mybir as mybir
import concourse.tile as tile
from concourse import bacc
from concourse.bass_utils import run_bass_kernel_spmd
from concourse.masks import make_identity

F32 = mybir.dt.float32
F32R = mybir.dt.float32r
BF16 = mybir.dt.bfloat16
AF = mybir.ActivationFunctionType
ALU = mybir.AluOpType

B, S, D = 2, 2048, 1024
H, HD = 16, 64
NCORES = 8
QB = 512          # query rows per core
ST = S // 128     # 16 s-tiles
QT = QB // 128    # 4 own s-tiles
EPS = 1e-5

def _build():
    nc = bacc.Bacc(None, target_bir_lowering=False)

    xT = nc.declare_dram_parameter("xT", [D, S], F32R, isOutput=False)
    Wq = nc.declare_dram_parameter("Wq", [D, D], F32R, isOutput=False)
    Wk = nc.declare_dram_parameter("Wk", [D, D], F32R, isOutput=False)
    Wv = nc.declare_dram_parameter("Wv", [D, D], F32R, isOutput=False)
    Wo = nc.declare_dram_parameter("Wo", [D, D], F32, isOutput=False)
    gq = nc.declare_dram_parameter("gq", [1, D], F32, isOutput=False)
    bq = nc.declare_dram_parameter("bq", [1, D], F32, isOutput=False)
    gk = nc.declare_dram_parameter("gk", [1, D], F32, isOutput=False)
    bk = nc.declare_dram_parameter("bk", [1, D], F32, isOutput=False)
    out = nc.declare_dram_parameter("out", [QB, D], F32, isOutput=True)

    with tile.TileContext(nc) as tc:
        with (
            tc.tile_pool(name="const", bufs=1) as cst,
            tc.tile_pool(name="qt", bufs=1) as qtp,
            tc.tile_pool(name="va", bufs=1) as vap,
            tc.tile_pool(name="dram", bufs=1, space="DRAM") as dram,
        ):
            ident = cst.tile([128, 128], F32)
            make_identity(nc, ident)
            ident_bf = cst.tile([128, 128], BF16)
            nc.vector.tensor_copy(ident_bf, ident)
            # gamma/beta rearranged to per-partition columns [128, 8]
            gq_c = cst.tile([128, 8], F32)
            bq_c = cst.tile([128, 8], F32)
            gk_c = cst.tile([128, 8], F32)
            bk_c = cst.tile([128, 8], F32)
            for t, p_ in ((gq, gq_c), (bq, bq_c), (gk, gk_c), (bk, bk_c)):
                nc.sync.dma_start(
                    out=p_, in_=t.ap().rearrange("o (i p) -> (o p) i", p=128)
                )
            ones16 = cst.tile([128, 16], F32)
            nc.vector.memset(ones16, 1.0)
            eps_t = cst.tile([128, 1], F32)
            nc.vector.memset(eps_t, EPS)

            qt_sb = qtp.tile([128, 8, QB], BF16)          # qT, d-block major
            kt0_sb = qtp.tile([128, S], BF16)             # pair-0 kT kept resident
            kT_dram = dram.tile([D, S], BF16)
            # v_aug [keys=128, h, hd+1] per keytile, ones column at 64
            va = [vap.tile([128, H, HD + 1], BF16, tag=f"va{m}", name=f"va{m}") for m in range(ST)]
            for m in range(ST):
                nc.vector.tensor_copy(va[m][:, :, HD], ones16)

            # ---------------- Stage A/B: projections + LN + transposes ----
            with (
                tc.tile_pool(name="w", bufs=1) as wp,
                tc.tile_pool(name="stA", bufs=3) as ap,
                tc.tile_pool(name="stA2", bufs=2) as ap2,
                tc.tile_pool(name="ps_tr", bufs=2, space="PSUM") as ps_tr,
                tc.tile_pool(name="ps_pj", bufs=3, space="PSUM") as ps_pj,
            ):
                wq_sb = wp.tile([128, 8, D], F32R)
                wk_sb = wp.tile([128, 8, D], F32R)
                wv_sb = wp.tile([128, 8, D], F32R)

                def load_w(w_par, w_sb):
                    for i in range(8):
                        nc.sync.dma_start(
                            out=w_sb[:, i, :],
                            in_=w_par.ap()[128 * i : 128 * (i + 1), :],
                        )

                def layer_norm_transpose(pre, g_c, b_c, sink):
                    """LN rows of pre [128, D], transpose, fuse gamma/beta into
                    the psum->sbuf copy; sink(i, psum_ap, g_col, b_col)."""
                    stats = ap.tile([128, 2, 6], F32, tag="bnst")
                    nc.vector.bn_stats(stats[:, 0, :], pre[:, 0:512])
                    nc.vector.bn_stats(stats[:, 1, :], pre[:, 512:1024])
                    mv = ap.tile([128, 2], F32, tag="bnmv")
                    nc.vector.bn_aggr(mv, stats)
                    # rstd = 1/sqrt(var+eps): ACT Sqrt + DVE recip seed, then
                    # one Newton step y1 = y0*(1.5 - 0.5*(var+eps)*y0^2).
                    ve = ap.tile([128, 1], F32, tag="ve")
                    nc.vector.tensor_scalar_add(ve, mv[:, 1:2], EPS)
                    s0 = ap.tile([128, 1], F32, tag="s0")
                    nc.scalar.activation(s0, mv[:, 1:2], AF.Sqrt, bias=eps_t)
                    y0 = ap.tile([128, 1], F32, tag="y0")
                    nc.vector.reciprocal(y0, s0)
                    t1 = ap.tile([128, 1], F32, tag="t1")
                    nc.vector.tensor_mul(t1, y0, y0)
                    nc.vector.tensor_mul(t1, t1, ve)
                    rstd = ap.tile([128, 1], F32, tag="rstd")
                    nc.vector.tensor_scalar(t1, t1, -0.5, 1.5, ALU.mult, ALU.add)
                    nc.vector.tensor_mul(rstd, t1, y0)
                    ln = ap.tile([128, D], BF16, tag="ln")
                    nc.vector.tensor_scalar(
                        ln, pre, mv[:, 0:1], rstd, ALU.subtract, ALU.mult
                    )
                    for i in range(8):
                        pt = ps_tr.tile([128, 128], BF16, tag="ptr")
                        nc.tensor.transpose(
                            pt, ln[:, 128 * i : 128 * (i + 1)], ident_bf
                        )
                        sink(i, pt, g_c[:, i : i + 1], b_c[:, i : i + 1])

                xT_r = xT.ap().rearrange("(i p) s -> p i s", p=128)
                for m in range(ST):
                    xT_s = ap.tile([128, 8, 128], F32R, tag="xT_s")
                    nc.sync.dma_start(
                        out=xT_s, in_=xT_r[:, :, 128 * m : 128 * (m + 1)]
                    )
                    if m == 0:
                        load_w(Wq, wq_sb)

                    def project(w_sb, n):
                        pp = ps_pj.tile([128, 512], F32, tag="pp")
                        for i in range(8):
                            nc.tensor.matmul(
                                pp,
                                lhsT=xT_s[:, i, :],
                                rhs=w_sb[:, i, 512 * n : 512 * (n + 1)],
                                start=(i == 0),
                                stop=(i == 7),
                            )
                        return pp

                    # q for own rows (rotated rows 0..511 = s-tiles 0..3)
                    if m < QT:
                        q_pre = ap2.tile([128, D], F32, tag="q_pre")
                        for n in range(2):
                            pp = project(wq_sb, n)
                            nc.scalar.copy(
                                q_pre[:, 512 * n : 512 * (n + 1)], pp
                            )

                        def q_sink(i, pt, g_col, b_col, m=m):
                            nc.vector.tensor_scalar(
                                qt_sb[:, i, 128 * m : 128 * (m + 1)],
                                pt, g_col, b_col, ALU.mult, ALU.add,
                            )
                        layer_norm_transpose(q_pre, gq_c, bq_c, q_sink)

                    # k (all rows)
                    if m == 0:
                        load_w(Wk, wk_sb)
                    k_pre = ap2.tile([128, D], F32, tag="k_pre")
                    for n in range(2):
                        pp = project(wk_sb, n)
                        nc.scalar.copy(k_pre[:, 512 * n : 512 * (n + 1)], pp)
                    kT_s = ap.tile([128, 7, 128], BF16, tag="kT_s")

                    def k_sink(i, pt, g_col, b_col, m=m):
                        dst = (
                            kt0_sb[:, 128 * m : 128 * (m + 1)]
                            if i == 0
                            else kT_s[:, i - 1, :]
                        )
                        nc.vector.tensor_scalar(
                            dst, pt, g_col, b_col, ALU.mult, ALU.add
                        )
                    layer_norm_transpose(k_pre, gk_c, bk_c, k_sink)
                    nc.sync.dma_start(
                        out=kT_dram.rearrange("(i p) s -> p i s", p=128)[
                            :, 1:8, 128 * m : 128 * (m + 1)
                        ],
                        in_=kT_s,
                    )

                    # v (all rows): straight into the resident v_aug tiles
                    if m == 0:
                        load_w(Wv, wv_sb)
                    for n in range(2):
                        pp = project(wv_sb, n)
                        nc.scalar.copy(
                            va[m][:, 8 * n : 8 * (n + 1), 0:HD],
                            pp.rearrange("p (h d) -> p h d", h=8),
                        )

            # ---------------- Stage C/D: attention + output projection ----
            with (
                tc.tile_pool(name="stC", bufs=2) as cp,
                tc.tile_pool(name="wo", bufs=1) as wop,
                tc.tile_pool(name="pt", bufs=4) as ptp,
                tc.tile_pool(name="ps_sc", bufs=3, space="PSUM") as ps_sc,
                tc.tile_pool(name="ps_pv", bufs=1, space="PSUM") as ps_pv,
            ):
                wo_sb = wop.tile([128, 8, D], BF16)
                for i in range(8):
                    wtm = cp.tile([128, D], F32, tag="wtm")
                    nc.sync.dma_start(
                        out=wtm, in_=Wo.ap()[128 * i : 128 * (i + 1), :]
                    )
                    nc.vector.tensor_copy(wo_sb[:, i, :], wtm)
                outT_sb = wop.tile([128, 8, QB], BF16)

                for p in range(8):  # head pairs (2p, 2p+1)
                    if p == 0:
                        ktp_t = kt0_sb
                    else:
                        ktp_t = cp.tile([128, S], BF16, tag="ktp")
                        nc.sync.dma_start(
                            out=ktp_t, in_=kT_dram[128 * p : 128 * (p + 1), :]
                        )
                    # SBUF accumulators for the two PV half-chains
                    pvs_a = cp.tile([HD + 1, 512], F32, tag="pvsa")
                    pvs_b = cp.tile([HD + 1, 512], F32, tag="pvsb")
                    for half in range(2):
                        pv_a = ps_pv.tile([128, 512], F32, tag="pva")
                        pv_b = ps_pv.tile([128, 512], F32, tag="pvb")
                        for g in range(4):  # keytile groups of 2
                            kt0 = 8 * half + 2 * g
                            sA = ps_sc.tile([128, 1024], F32, tag="sc", name="sA")
                            sB = ps_sc.tile([128, 1024], F32, tag="sc", name="sB")
                            for e in range(2):
                                kt = kt0 + e
                                nc.tensor.matmul(
                                    sA[:, 512 * e : 512 * (e + 1)],
                                    lhsT=ktp_t[0:64, 128 * kt : 128 * (kt + 1)],
                                    rhs=qt_sb[0:64, p, :],
                                    start=True, stop=True,
                                    tile_position=(0, 0),
                                )
                                nc.tensor.matmul(
                                    sB[:, 512 * e : 512 * (e + 1)],
                                    lhsT=ktp_t[64:128, 128 * kt : 128 * (kt + 1)],
                                    rhs=qt_sb[64:128, p, :],
                                    start=True, stop=True,
                                    tile_position=(64, 0),
                                )
                            pA = ptp.tile([128, 1024], BF16, tag="pA")
                            pB = ptp.tile([128, 1024], BF16, tag="pB")
                            nc.scalar.activation(pA, sA, AF.Exp)
                            nc.scalar.activation(pB, sB, AF.Exp)
                            for e in range(2):
                                kt = kt0 + e
                                nc.tensor.matmul(
                                    pv_a[0 : HD + 1, :],
                                    lhsT=va[kt][:, 2 * p, :],
                                    rhs=pA[:, 512 * e : 512 * (e + 1)],
                                    start=(g == 0 and e == 0),
                                    stop=(g == 3 and e == 1),
                                    skip_group_check=True,
                                )
                                nc.tensor.matmul(
                                    pv_b[0 : HD + 1, :],
                                    lhsT=va[kt][:, 2 * p + 1, :],
                                    rhs=pB[:, 512 * e : 512 * (e + 1)],
                                    start=(g == 0 and e == 0),
                                    stop=(g == 3 and e == 1),
                                    skip_group_check=True,
                                )
                        if half == 0:
                            nc.vector.tensor_copy(pvs_a, pv_a[0 : HD + 1, :])
                            nc.vector.tensor_copy(pvs_b, pv_b[0 : HD + 1, :])
                        else:
                            nc.vector.tensor_add(pvs_a, pvs_a, pv_a[0 : HD + 1, :])
                            nc.vector.tensor_add(pvs_b, pvs_b, pv_b[0 : HD + 1, :])
                    # normalize by the fused row sums, write into outT
                    r_a = cp.tile([1, 512], F32, tag="r_a")
                    nc.vector.reciprocal(r_a, pvs_a[HD : HD + 1, :])
                    rb_a = cp.tile([64, 512], F32, tag="rb_a")
                    nc.gpsimd.partition_broadcast(rb_a, r_a)
                    nc.vector.tensor_tensor(
                        outT_sb[0:64, p, :], pvs_a[0:HD, :], rb_a, ALU.mult
                    )
                    r_b = cp.tile([1, 512], F32, tag="r_b")
                    nc.vector.reciprocal(r_b, pvs_b[HD : HD + 1, :])
                    rb_b = cp.tile([64, 512], F32, tag="rb_b")
                    nc.gpsimd.partition_broadcast(rb_b, r_b)
                    tmp_b = cp.tile([64, 512], BF16, tag="tmp_b")
                    nc.vector.tensor_tensor(tmp_b, pvs_b[0:HD, :], rb_b, ALU.mult)
                    nc.sync.dma_start(out=outT_sb[64:128, p, :], in_=tmp_b)

                # output projection: out[u*128:, :] = outT^T @ Wo
                for u in range(QT):
                    for n in range(2):
                        po = ps_pv.tile(
                            [128, 512], F32, tag=("pva" if (u + n) % 2 == 0 else "pvb"),
                            name="po",
                        )
                        for i in range(8):
                            nc.tensor.matmul(
                                po,
                                lhsT=outT_sb[:, i, 128 * u : 128 * (u + 1)],
                                rhs=wo_sb[:, i, 512 * n : 512 * (n + 1)],
                                start=(i == 0),
                                stop=(i == 7),
                            )
                        oo = cp.tile([128, 512], F32, tag="oo")
                        nc.vector.tensor_copy(oo, po)
                        nc.sync.dma_start(
                            out=out.ap()[
                                128 * u : 128 * (u + 1), 512 * n : 512 * (n + 1)
                            ],
                            in_=oo,
                        )

    nc.compile()
    return nc


_NC_CACHE = {}


def _get_nc():
    if "nc" not in _NC_CACHE:
        _NC_CACHE["nc"] = _build()
    return _NC_CACHE["nc"]


def _install_trace_hook():
    """Best-effort registration of the axon NTFF profiling hook."""
    import sys, types

    if "antenv.axon_hooks" in sys.modules:
        return
    try:
        import antenv  # noqa: F401
        from trn_agent_boot.trn_boot import _ntff_profile_via_ctypes

        mod = types.ModuleType("antenv.axon_hooks")
        _h = [None]
        mod.set_axon_ntff_profile_hook = lambda h: _h.__setitem__(0, h)
        mod.get_axon_ntff_profile_hook = lambda: _h[0]
        sys.modules["antenv.axon_hooks"] = mod
        antenv.axon_hooks = mod
        mod.set_axon_ntff_profile_hook(
            _ntff_profile_via_ctypes("/opt/axon/libaxon_pjrt.so")
        )
    except Exception:
        pass


def kernel(_trace=False, **inputs):
    x = np.asarray(inputs["x"], dtype=np.float32)
    assert x.shape == (B, S, D)
    weights = {
        k: np.ascontiguousarray(np.asarray(inputs[k], dtype=np.float32))
        for k in ("Wq", "Wk", "Wv", "Wo")
    }
    vecs = {
        "gq": inputs["q_gamma"], "bq": inputs["q_beta"],
        "gk": inputs["k_gamma"], "bk": inputs["k_beta"],
    }
    vecs = {
        k: np.ascontiguousarray(np.asarray(v, dtype=np.float32)).reshape(1, D)
        for k, v in vecs.items()
    }

    xT_full = [np.ascontiguousarray(x[b].T) for b in range(B)]
    in_maps = []
    for c in range(NCORES):
        b, j = divmod(c, 4)
        xT = xT_full[b]
        if j:
            xT = np.ascontiguousarray(
                np.concatenate([xT[:, QB * j :], xT[:, : QB * j]], axis=1)
            )
        m = {"xT": xT}
        m.update(weights)
        m.update(vecs)
        in_maps.append(m)

    if _trace:
        _install_trace_hook()
    nc = _get_nc()
    res = run_bass_kernel_spmd(
        nc, in_maps, core_ids=list(range(NCORES)), trace=_trace
    )

    out = np.empty((B, S, D), dtype=np.float32)
    for c in range(NCORES):
        b, j = divmod(c, 4)
        out[b, QB * j : QB * (j + 1)] = res.results[c]["out"]

    if _trace:
        kernel.last_results = res
    return out
